# revision 54
# baseline (speedup 1.0000x reference)
import sys

sys.path.insert(0, "/opt/trn_rl_repo")
import numpy as np
import concourse.bass as bass
import concourse.bacc as bacc
import concourse.mybir as mybir
from concourse.tile import TileContext
from concourse.bass import ts

F32 = mybir.dt.float32
F16 = mybir.dt.float16

# Problem dims (hardcoded per spec)
N, D, H, W, C = 2, 32, 32, 32, 128
G, Cg, K = 8, 16, 27
OM = 864  # G * K * 4
L = D * H * W

# Sharding: 8 cores = (n in 2) x (d-block in 4); 8 own d-slices per core.
NCORES = 8
DB = 8          # own d slices per core
DL = DB + 4     # slab depth with halo 2 on each side
HP, WP = H + 4, W + 4   # padded h/w (halo 2) for shift addressing
SLAB = DL * H * W        # 12288 input slab voxels
VOWN = DB * H * W        # 8192 own voxels
PADVOL = DL * HP * WP    # padded v volume free size

# Tuning knobs
import os
NPOOL_PACKS = int(os.environ.get("KNPOOL_PACKS", "0"))  # F 5-packs per slice on gpsimd
NPOOL_T3 = int(os.environ.get("KNPOOL_T3", "1"))        # T3 ops per block on gpsimd
NPOOL_P1 = int(os.environ.get("KNPOOL_P1", "0"))        # P1 op on gpsimd
NPOOL_MWW = int(os.environ.get("KNPOOL_MWW", "1"))      # MWW op on gpsimd
SADDS_PE = os.environ.get("KSADDS_PE", "1") == "1"      # scatter-adds on PE vs DVE
SREP_Q = int(os.environ.get("KSREP_Q", "2"))            # srep queue pattern id
KDROP = int(os.environ.get("KDROP", "0"))               # drop low-mass (Dd,Dh) pairs
KDROPW = int(os.environ.get("KDROPW", "0"))             # drop outer dw taps (5->3)

_NC_CACHE = {}


def _perm_rows():
    """Permute offmask rows: per group [od(27), oh(27), ow(27), m(27)]."""
    perm = np.zeros(OM, dtype=np.int64)
    p = 0
    for g in range(G):
        base = g * 108
        for axis in range(3):
            for k in range(K):
                perm[p] = base + k * 3 + axis
                p += 1
        for k in range(K):
            perm[p] = base + 81 + k
            p += 1
    return perm


def build_nc():
    nc = bacc.Bacc("TRN2", target_bir_lowering=False, debug=False, num_devices=8)
    xTd = nc.declare_dram_parameter("xTd", [C, SLAB], F16, isOutput=False)
    vwT = nc.declare_dram_parameter("vwT", [C, C], F16, isOutput=False)
    vb = nc.declare_dram_parameter("vb", [C, 1], F32, isOutput=False)
    vbdiag = nc.declare_dram_parameter("vbdiag", [C, C], F16, isOutput=False)
    omT = nc.declare_dram_parameter("omT", [C, OM], F16, isOutput=False)
    ombias = nc.declare_dram_parameter("ombias", [128, OM], F16, isOutput=False)
    onesr0 = nc.declare_dram_parameter("onesr0", [128, 128], F16, isOutput=False)
    owT = nc.declare_dram_parameter("owT", [C, C], F16, isOutput=False)
    ob = nc.declare_dram_parameter("ob", [C, 1], F32, isOutput=False)
    idin = nc.declare_dram_parameter("idin", [128, 128], F16, isOutput=False)
    out_d = nc.declare_dram_parameter("out", [VOWN, C], F32, isOutput=True)

    RELU = mybir.ActivationFunctionType.Relu
    ABS = mybir.ActivationFunctionType.Abs
    IDENT = mybir.ActivationFunctionType.Identity

    with TileContext(nc) as tc:
        with (
            tc.tile_pool(name="const", bufs=1) as cpool,
            tc.tile_pool(name="big", bufs=1) as bigpool,
            tc.tile_pool(name="ps_tr", bufs=2, space="PSUM") as ps_tr,
            tc.tile_pool(name="ps_om", bufs=1, space="PSUM") as ps_om,
            tc.tile_pool(name="ps_s", bufs=1, space="PSUM") as ps_s,
            tc.tile_pool(name="ps_out", bufs=1, space="PSUM") as ps_out_pool,
            tc.tile_pool(name="sbuild", bufs=1) as spool,
            tc.tile_pool(name="stp", bufs=2) as stpool,
            tc.tile_pool(name="srep", bufs=int(os.environ.get("KREPBUFS", "6"))) as reppool,
            tc.tile_pool(name="prod", bufs=int(os.environ.get("KPRODBUFS", "2"))) as prodpool,
            tc.tile_pool(name="outp", bufs=1) as opool,
        ):
            ident16 = cpool.tile([128, 128], F16)
            nc.sync.dma_start(out=ident16[:], in_=idin[:])
            vw_sb = cpool.tile([C, C], F16)
            nc.sync.dma_start(out=vw_sb[:], in_=vwT[:])
            om_w = cpool.tile([C, OM], F16)
            nc.sync.dma_start(out=om_w[:], in_=omT[:])
            omb_sb = cpool.tile([128, OM], F16)
            nc.sync.dma_start(out=omb_sb[:], in_=ombias[:])
            ones_sb = cpool.tile([128, 128], F16)
            nc.sync.dma_start(out=ones_sb[:], in_=onesr0[:])
            ow_sb = cpool.tile([C, C], F16)
            nc.sync.dma_start(out=ow_sb[:], in_=owT[:])
            vb_sb = cpool.tile([C, 1], F32)
            nc.sync.dma_start(out=vb_sb[:], in_=vb[:])
            ob_sb = cpool.tile([C, 1], F32)
            nc.sync.dma_start(out=ob_sb[:], in_=ob[:])
            cone = cpool.tile([128, 1], F32)
            nc.gpsimd.memset(cone[:], 1.0)
            cneg = cpool.tile([128, 1], F32)
            nc.gpsimd.memset(cneg[:], -1.0)
            zeros16 = cpool.tile([128, 512], F16)
            nc.gpsimd.memset(zeros16[:], 0.0)
            ones512 = cpool.tile([128, 512], F16)
            nc.gpsimd.memset(ones512[:], 1.0)
            vbd_sb = cpool.tile([C, C], F16)
            nc.sync.dma_start(out=vbd_sb[:], in_=vbdiag[:])

            # ---- load pre-transposed input [ci, SLAB] f16 (4 chunked DMAs) ----
            xT = bigpool.tile([C, SLAB], F16)
            for q in range(4):
                nc.sync.dma_start(out=xT[:, ts(q, SLAB // 4)], in_=xTd[:, ts(q, SLAB // 4)])

            # ---- Stage B: value proj -> v_pad f16 [c, PADVOL] ----
            v_pad = bigpool.tile([C, PADVOL], F16)
            nc.gpsimd.memset(v_pad[:], 0.0)

            def stage_b(ts_range=None):
                for t in (ts_range if ts_range is not None else range(SLAB // 512)):
                    psb = ps_out_pool.tile([128, 1024], F32, tag="acc")
                    ps = psb[:, 0:512]
                    nc.tensor.matmul(ps, vbd_sb[:], ones512[:], start=True, stop=False)
                    nc.tensor.matmul(ps, vw_sb[:], xT[:, ts(t, 512)], start=False, stop=True)
                    d_loc, h0 = t // 2, (t % 2) * 16
                    off = (d_loc * HP + h0 + 2) * WP + 2
                    dst = bass.AP(v_pad.tensor, v_pad.offset + off, [v_pad.ap[0], [WP, 16], [1, W]])
                    src = bass.AP(ps.tensor, ps.offset, [ps.ap[0], [32, 16], [1, 32]])
                    if t % 2 == 0:
                        nc.vector.tensor_copy(out=dst, in_=src)
                    else:
                        nc.scalar.activation(out=dst, in_=src, func=IDENT, scale=1.0)

            # per-slice loop (software-pipelined: build slice s+1 before applying s)
            STs = {}

            T3s = {}

            def build_front(s, b):
                if True:
                    if b == 0:
                        ST = stpool.tile([128, 8 * 1024], F16, tag="ST")
                        STs[s] = ST
                    ST = STs[s]
                    blk = s * 8 + b
                    xsl = xT[:, 2048 + 128 * blk:2048 + 128 * blk + 128]
                    pso = ps_om.tile([128, 1024], F32, tag="om")
                    # init psum with bias (broadcast via ones-row matmul), accumulate om proj
                    nc.tensor.matmul(pso[:, 0:432], ones_sb[:], omb_sb[:, 0:432], start=True, stop=False)
                    nc.tensor.matmul(pso[:, 512:944], ones_sb[:], omb_sb[:, 432:864], start=True, stop=False)
                    nc.tensor.matmul(pso[:, 0:432], xsl, om_w[:, 0:432], start=False, stop=True)
                    nc.tensor.matmul(pso[:, 512:944], xsl, om_w[:, 432:864], start=False, stop=True)

                    # W3 [3v,3a,8g,27k] layout: v stride 648, axis 216, k 8, g 1
                    # tents read om directly from PSUM; g split across banks as (gh:512, gl:108)
                    W3 = spool.tile([128, 1944], F16, tag="W3")
                    tmp = spool.tile([128, 648], F16, tag="tmp")
                    m16 = spool.tile([128, 216], F16, tag="m16")
                    if s == 0:
                        # prologue: stage om in SBUF so pso frees after 2 copies, not 5 tent reads
                        oms = spool.tile([128, OM], F32, tag="om_sbuf")
                        nc.scalar.activation(out=oms[:, 0:432], in_=pso[:, 0:432], func=IDENT, scale=1.0)
                        nc.scalar.activation(out=oms[:, 432:864], in_=pso[:, 512:944], func=IDENT, scale=1.0)
                        in_offs = bass.AP(oms.tensor, oms.offset, [oms.ap[0], [108, 8], [27, 3], [1, 27]])
                        m_in = bass.AP(oms.tensor, oms.offset + 81, [oms.ap[0], [108, 8], [1, 27]])
                        out_gak = lambda v: bass.AP(W3.tensor, W3.offset + v * 648, [W3.ap[0], [1, 8], [216, 3], [8, 27]])
                        tmp_gak = bass.AP(tmp.tensor, tmp.offset, [tmp.ap[0], [1, 8], [216, 3], [8, 27]])
                        m_out = bass.AP(m16.tensor, m16.offset, [m16.ap[0], [1, 8], [8, 27]])
                    else:
                        in_offs = bass.AP(pso.tensor, pso.offset, [pso.ap[0], [512, 2], [108, 4], [27, 3], [1, 27]])
                        m_in = bass.AP(pso.tensor, pso.offset + 81, [pso.ap[0], [512, 2], [108, 4], [1, 27]])
                        out_gak = lambda v: bass.AP(W3.tensor, W3.offset + v * 648, [W3.ap[0], [4, 2], [1, 4], [216, 3], [8, 27]])
                        tmp_gak = bass.AP(tmp.tensor, tmp.offset, [tmp.ap[0], [4, 2], [1, 4], [216, 3], [8, 27]])
                        m_out = bass.AP(m16.tensor, m16.offset, [m16.ap[0], [4, 2], [1, 4], [8, 27]])
                    nc.scalar.activation(out=out_gak(2), in_=in_offs, func=RELU, scale=1.0)
                    nc.scalar.activation(out=out_gak(0), in_=in_offs, func=RELU, scale=cneg[:])
                    nc.scalar.activation(out=tmp_gak, in_=in_offs, func=ABS, scale=1.0)
                    nc.scalar.activation(out=out_gak(1), in_=tmp_gak, func=IDENT, scale=cneg[:], bias=cone[:])
                    nc.scalar.activation(out=m_out, in_=m_in, func=IDENT, scale=1.0)
                    # MWW [3dw, (k,g)216] = W3[dw, a=2, k, g] * m[k, g]
                    MWW = spool.tile([128, 648], F16, tag="MWW")
                    (nc.gpsimd if (NPOOL_MWW and (s > 0 or b < int(os.environ.get("KP0B", "0")))) else nc.vector).tensor_tensor(
                        out=bass.AP(MWW.tensor, MWW.offset, [MWW.ap[0], [216, 3], [1, 216]]),
                        in0=bass.AP(W3.tensor, W3.offset + 432, [W3.ap[0], [648, 3], [1, 216]]),
                        in1=bass.AP(m16.tensor, m16.offset, [m16.ap[0], [0, 3], [1, 216]]),
                        op=mybir.AluOpType.mult)
                    # P1 [3dd,3dh,(k,g)] = W3[dd,0,k,g]*W3[dh,1,k,g]
                    P1 = spool.tile([128, 1944], F16, tag="P1")
                    (nc.gpsimd if NPOOL_P1 else nc.vector).tensor_tensor(
                        out=bass.AP(P1.tensor, P1.offset, [P1.ap[0], [648, 3], [216, 3], [1, 216]]),
                        in0=bass.AP(W3.tensor, W3.offset, [W3.ap[0], [648, 3], [0, 3], [1, 216]]),
                        in1=bass.AP(W3.tensor, W3.offset + 216, [W3.ap[0], [0, 3], [648, 3], [1, 216]]),
                        op=mybir.AluOpType.mult)
                    # T3 [3dd,3dh,3dw,(k,g)]: 3 ops (per dw)
                    T3 = spool.tile([128, 5832], F16, tag="T3")
                    for dw in range(3):
                        eng = nc.gpsimd if (dw < NPOOL_T3 and (s > 0 or b < int(os.environ.get("KP0B", "0")))) else nc.vector
                        eng.tensor_tensor(
                            out=bass.AP(T3.tensor, T3.offset + dw * 216, [T3.ap[0], [1944, 3], [648, 3], [1, 216]]),
                            in0=bass.AP(P1.tensor, P1.offset, [P1.ap[0], [648, 3], [216, 3], [1, 216]]),
                            in1=bass.AP(MWW.tensor, MWW.offset + dw * 216, [MWW.ap[0], [0, 3], [0, 3], [1, 216]]),
                            op=mybir.AluOpType.mult)

                    T3s[(s, b)] = (T3, ST)

            def build_back(s, b):
                if True:
                    T3, ST = T3s.pop((s, b))
                    # S [vox, (delta 125, g 8)] with delta lin (Dd*5+Dh)*5+Dw
                    if SADDS_PE:
                        psS = ps_s.tile([128, 1024], F32, tag="psS")
                        # zero both banks via zero-weight matmuls
                        nc.tensor.matmul(psS[:, 0:512], zeros16[:, 0:128], zeros16[:, 0:512], start=True, stop=False)
                        nc.tensor.matmul(psS[:, 512:1024], zeros16[:, 0:128], zeros16[:, 0:512], start=True, stop=False)
                        for dd in range(3):
                            for dh in range(3):
                                for dw in range(3):
                                    soff = (dd * 25 + dh * 5 + dw) * 8
                                    t3off = (dd * 9 + dh * 3 + dw) * 216
                                    outap = bass.AP(psS.tensor, psS.offset + soff,
                                                    [psS.ap[0], [200, 3], [40, 3], [8, 3], [1, 8]])
                                    rhs = bass.AP(T3.tensor, T3.offset + t3off,
                                                  [T3.ap[0], [72, 3], [24, 3], [8, 3], [1, 8]])
                                    last = (dd == 2 and dh == 2 and dw == 2)
                                    nc.tensor.matmul(outap, ident16[:], rhs, start=False, stop=last)
                        S16 = spool.tile([128, 1024], F16, tag="S16")
                        nc.scalar.activation(out=S16[:, 0:1000], in_=psS[:, 0:1000], func=IDENT, scale=1.0)
                    else:
                        S16 = spool.tile([128, 1024], F16, tag="S16")
                        nc.gpsimd.memset(S16[:], 0.0)
                        for dd in range(3):
                            for dh in range(3):
                                for dw in range(3):
                                    soff = (dd * 25 + dh * 5 + dw) * 8
                                    t3off = (dd * 9 + dh * 3 + dw) * 216
                                    sap = bass.AP(S16.tensor, S16.offset + soff,
                                                  [S16.ap[0], [200, 3], [40, 3], [8, 3], [1, 8]])
                                    nc.vector.tensor_tensor(
                                        out=sap, in0=sap,
                                        in1=bass.AP(T3.tensor, T3.offset + t3off,
                                                    [T3.ap[0], [72, 3], [24, 3], [8, 3], [1, 8]]),
                                        op=mybir.AluOpType.add)

                    # ---- Stage E: transpose S16 -> ST [16dsub x 8g, vox] f16 ----
                    psT = ps_tr.tile([128, 1024], F16, tag="tr")
                    for t in range(8):
                        inap = bass.AP(S16.tensor, S16.offset + 128 * t, [S16.ap[0], [1, 128]])
                        nc.tensor.matmul(psT[:, ts(t, 128)], inap, ident16[:], is_transpose=True)
                    # one wide copy to ST, strided dst (8 chunks at b*128 within 1024-blocks)
                    nc.scalar.activation(
                        out=bass.AP(ST.tensor, ST.offset + b * 128, [ST.ap[0], [1024, 8], [1, 128]]),
                        in_=psT[:], func=IDENT, scale=1.0)

            def apply_slice(s):
                # ---- Stage F: 25 packs of 5 w-shifts; DVE mult, PE accumulates out-proj
                # build blocks of slice s+1 interleaved between packs ----
                ST = STs[s]
                pso_acc = ps_out_pool.tile([128, 1024], F32, tag="acc")
                packs = []
                for pack in range(25):
                    Dd, Dh = pack // 5, pack % 5
                    oud, ouh = Dd in (0, 4), Dh in (0, 4)
                    if KDROP == 1 and oud and ouh:
                        continue
                    if KDROP == 2 and (oud or ouh):
                        continue
                    if KDROP == 3 and oud:
                        continue
                    packs.append(pack)
                dws = (1, 2, 3) if KDROPW else (0, 1, 2, 3, 4)
                import os as _os
                ILV = int(_os.environ.get("KILV", "2"))
                BOFF = int(_os.environ.get("KBOFF", "3"))
                for ip, pack in enumerate(packs):
                    if s + 1 < DB:
                        if ILV > 0:
                            if ip % ILV == 0 and ip // ILV < 8:
                                build_front(s + 1, ip // ILV)
                            if (ip - BOFF) >= 0 and (ip - BOFF) % ILV == 0 and (ip - BOFF) // ILV < 8:
                                build_back(s + 1, (ip - BOFF) // ILV)
                        elif ILV == 0 and ip == 0:
                            for b in range(8):
                                build_front(s + 1, b)
                                build_back(s + 1, b)
                    Dd, Dh = pack // 5, pack % 5
                    srep5 = reppool.tile([128, 5120], F16, tag="srep")
                    for dw in dws:
                        delta = (Dd * 5 + Dh) * 5 + dw
                        t, dsub = delta // 16, delta % 16
                        src = bass.AP(ST.tensor, ST.offset + t * 1024 + dsub * 8 * ST.ap[0][0],
                                      [[ST.ap[0][0], 8], [0, 16], [1, 1024]])
                        qsel = SREP_Q if (s < DB - 1 or os.environ.get("KLASTQ", "3") == "0") else int(os.environ.get("KLASTQ", "3"))
                        if ip >= int(os.environ.get("KTAILQ", "99")):
                            qsel = 3
                        pat = {2: (nc.sync, nc.scalar),
                               3: (nc.sync, nc.gpsimd),
                               4: (nc.sync, nc.gpsimd, nc.sync, nc.scalar, nc.gpsimd),
                               5: (nc.sync, nc.scalar, nc.sync, nc.gpsimd, nc.scalar),
                               6: (nc.sync, nc.scalar, nc.gpsimd, nc.sync, nc.scalar),
                               7: (nc.sync, nc.scalar, nc.gpsimd, nc.scalar, nc.sync)}[qsel]
                        eng = pat[delta % len(pat)]
                        eng.dma_start(out=srep5[:, ts(dw, 1024)], in_=src)
                    P5 = prodpool.tile([128, 5120], F16, tag="P5")
                    voff = ((s + Dd) * HP + Dh) * WP
                    vap = bass.AP(v_pad.tensor, v_pad.offset + voff,
                                  [v_pad.ap[0], [1, 5], [WP, 32], [1, 32]])
                    npp = NPOOL_PACKS + (int(os.environ.get("KPOOL_LAST", "0")) if s == DB - 1 else 0)
                    eng = nc.gpsimd if pack >= 25 - npp else nc.vector
                    nw = len(dws)
                    KPOOL1 = int(os.environ.get("KPOOL1", "0"))
                    split1 = (nw == 5 and eng is nc.vector and ip % 2 == 1 and ip // 2 < KPOOL1)
                    if split1:
                        nc.vector.tensor_tensor(
                            out=bass.AP(P5.tensor, P5.offset, [P5.ap[0], [1024, 4], [1, 1024]]),
                            in0=bass.AP(vap.tensor, vap.offset, [v_pad.ap[0], [1, 4], [WP, 32], [1, 32]]),
                            in1=bass.AP(srep5.tensor, srep5.offset, [srep5.ap[0], [1024, 4], [1, 1024]]),
                            op=mybir.AluOpType.mult)
                        nc.gpsimd.tensor_tensor(
                            out=bass.AP(P5.tensor, P5.offset + 4096, [P5.ap[0], [1, 1024]]),
                            in0=bass.AP(vap.tensor, vap.offset + 4, [v_pad.ap[0], [WP, 32], [1, 32]]),
                            in1=bass.AP(srep5.tensor, srep5.offset + 4096, [srep5.ap[0], [1, 1024]]),
                            op=mybir.AluOpType.mult)
                    else:
                        eng.tensor_tensor(
                            out=bass.AP(P5.tensor, P5.offset + dws[0] * 1024, [P5.ap[0], [1024, nw], [1, 1024]]),
                            in0=bass.AP(vap.tensor, vap.offset + dws[0], [v_pad.ap[0], [1, nw], [WP, 32], [1, 32]]),
                            in1=bass.AP(srep5.tensor, srep5.offset + dws[0] * 1024, [srep5.ap[0], [1024, nw], [1, 1024]]),
                            op=mybir.AluOpType.mult)
                    for dw in dws:
                        first = (pack == packs[0] and dw == dws[0])
                        last = (pack == packs[-1] and dw == dws[-1])
                        nc.tensor.matmul(pso_acc[:, 0:512], ow_sb[:], P5[:, dw * 1024:dw * 1024 + 512],
                                         start=first, stop=last)
                        nc.tensor.matmul(pso_acc[:, 512:1024], ow_sb[:], P5[:, dw * 1024 + 512:dw * 1024 + 1024],
                                         start=first, stop=last)

                # ---- Stage G: bias, transpose, store ----
                yT = opool.tile([128, 1024], F16, tag="yT")
                nc.scalar.activation(out=yT[:, 0:512], in_=pso_acc[:, 0:512], func=IDENT, bias=ob_sb[:], scale=1.0)
                nc.scalar.activation(out=yT[:, 512:1024], in_=pso_acc[:, 512:1024], func=IDENT, bias=ob_sb[:], scale=1.0)
                obuf = opool.tile([128, 1024], F32, tag="obuf")
                psg = ps_tr.tile([128, 1024], F16, tag="tr")
                for t in range(8):
                    nc.tensor.matmul(psg[:, ts(t, 128)], yT[:, ts(t, 128)], ident16[:], is_transpose=True)
                nc.scalar.activation(out=obuf[:], in_=psg[:], func=IDENT, scale=1.0)
                # one DMA per slice: obuf [128, (t8,c128)] -> out rows [s*1024 .. +1024)
                oap = out_d[:]
                dst = bass.AP(oap.tensor, oap.offset + s * 1024 * C,
                              [[C, 128], [C * 128, 8], [1, C]])
                (nc.gpsimd if os.environ.get("KOUTQ", "0") == "1" else nc.sync).dma_start(out=dst, in_=obuf[:])

            if os.environ.get("KBILV", "1") == "1":
                KBN = int(os.environ.get("KBN", "3"))
                for b in range(8):
                    build_front(0, b)
                    build_back(0, b)
                    stage_b(range(KBN * b, min(KBN * b + KBN, 24)))
                stage_b(range(KBN * 8, 24))
            else:
                for b in range(8):
                    build_front(0, b)
                    build_back(0, b)
                stage_b()
            for s in range(DB):
                apply_slice(s)
                STs.pop(s)
    nc.compile()
    return nc


def prep_inputs(input, value_w, value_b, offmask_w, offmask_b, out_w, out_b):
    perm = _perm_rows()
    vwT = np.ascontiguousarray(np.asarray(value_w, np.float32).T.astype(np.float16))
    vb = np.asarray(value_b, np.float32).reshape(C, 1)
    vbdiag = np.ascontiguousarray(np.diag(np.asarray(value_b, np.float32)).astype(np.float16))
    omT = np.ascontiguousarray(np.asarray(offmask_w, np.float32)[perm].T.astype(np.float16))
    ombias = np.zeros((128, OM), np.float16)
    ombias[0, :] = np.asarray(offmask_b, np.float32)[perm].astype(np.float16)
    onesr0 = np.zeros((128, 128), np.float16)
    onesr0[0, :] = 1.0
    owT = np.ascontiguousarray(np.asarray(out_w, np.float32).T.astype(np.float16))
    ob = np.asarray(out_b, np.float32).reshape(C, 1)
    ident = np.eye(128, dtype=np.float16)
    xf = np.asarray(input, np.float32).reshape(N, D, H * W * C)
    in_maps = []
    for core in range(NCORES):
        n, db = core // 4, core % 4
        d0 = db * DB
        slab = np.zeros((DL, H * W * C), np.float32)
        lo, hi = max(0, d0 - 2), min(D, d0 + DB + 2)
        slab[lo - (d0 - 2):hi - (d0 - 2)] = xf[n, lo:hi]
        xTd = np.ascontiguousarray(slab.reshape(SLAB, C).T.astype(np.float16))
        in_maps.append(dict(xTd=xTd, vwT=vwT, vb=vb, vbdiag=vbdiag, omT=omT, ombias=ombias,
                            onesr0=onesr0, owT=owT, ob=ob, idin=ident))
    return in_maps


def kernel(**inputs):
    from concourse.bass_utils import run_bass_kernel_spmd
    if "nc" not in _NC_CACHE:
        _NC_CACHE["nc"] = build_nc()
    nc = _NC_CACHE["nc"]
    in_maps = prep_inputs(**inputs)
    res = run_bass_kernel_spmd(nc, in_maps, list(range(NCORES))).results
    out = np.zeros((N, D, H, W, C), np.float32)
    for core in range(NCORES):
        n, db = core // 4, core % 4
        out[n, db * DB:(db + 1) * DB] = res[core]["out"].reshape(DB, H, W, C)
    return out
